# revision 33
# baseline (speedup 1.0000x reference)
"""Per-sample depthwise 7x7 SAME cross-correlation on 8 trn2 NeuronCores.

Problem: inputs [32,128,128,128] (B,H,W,C), kernels [32,7,7,C].
out[b,h,w,c] = sum_{i,j} inputs[b, h+i-3, w+j-3, c] * kernels[b,i,j,c]

Strategy (pure data parallel, batch sharded 4 samples/core), v2:
  All 49 taps run on the TensorEngine as BANDED-TOEPLITZ matmuls --
  ~7 MACs per moving column instead of the 1 a diagonal stationary
  gives, so the PE does the whole conv at ~7x its diag rate and the
  elementwise engines only evacuate PSUM.

  - Partition space p = (ci, ws): 4 channels x 32 width positions.
    The moving tile per (b, cgroup-of-4) is [128, 5 wblk, 134 hpad]
    bf16: five OVERLAPPING width blocks (stride 26, halo 3 each side,
    zero-padded outside the image) so no cross-block seams exist;
    only block outputs ws in [3, 29) are valid and the host discards
    the rest. H padding (3+128+3) lives in the free dim: kernel row i
    is a free-dim slice [i : i+128] -- shifts are free.
  - Stationary per (b, i, cgroup): S[p=(ci,ws), o=(co,wo)] =
    (ci==co) * w[b, i, ws-wo+3, c] -- block-diagonal of four 32x32
    7-banded Toeplitz blocks. Engines cannot write per-partition
    -offset diagonals at rate, so the 896 stationaries per core are
    PREBUILT BY THE HOST and shipped as an extra 28.7 MB input --
    DMA is the one resource with headroom (total in+S+out ~72 MB at
    ~330 GB/s ~ 215 us, balanced against ~245 us of PE).
  - Per (b, cg): 7 accumulation matmuls into PSUM P0 [128,4,128] f32
    (one full bank, N=512) + 7 into P1 [128,128] (N=128), start/stop
    framing the i-group. ldweights (128 cols, FWL-eligible) hides
    behind the 213 ns N=512 matmuls.
  - Act evacuates PSUM to bf16 SBUF (f32 accumulation throughout; only
    the final store rounds to bf16: measured ~4e-3 max rel err vs the
    f32 reference, gate 2e-2), DMA out 21 MB bf16; host casts to f32
    and reassembles valid slices.

  Rejected: on-chip Toeplitz construction (no engine can write
  per-partition-offset diagonals faster than ~1 full-tile pass per
  stationary; gather/scatter live on GpSimd at ~2 ns/elem), Winograd
  (bf16 transform conditioning blows the 2e-2 gate), FFT (freq dims
  need >128 partitions), elementwise tap lanes (engine-sum floor
  ~650 us -- the v1 kernel at 890 us was already near it).
"""

import numpy as np
import ml_dtypes

import concourse.bass as bass
import concourse.tile as tile
from concourse import bacc, mybir
from concourse.bass_utils import run_bass_kernel_spmd

B, H, W, C = 32, 128, 128, 128
KH = KW = 7
PAD = 3
N_CORES = 8
BPC = B // N_CORES          # samples per core
HP = H + 2 * PAD            # 134 padded rows (free dim)
CSUB = 4                    # channels per partition tile
WSUB = 32                   # width positions per channel in partitions
NCG = C // CSUB             # 32 channel groups
VAL = WSUB - 2 * PAD        # 26 valid outputs per width block
NBLK = -(-W // VAL)         # 5 overlapping width blocks (stride VAL)

_PROGRAM_CACHE = {}


def _build_program():
    f32 = mybir.dt.float32
    bf16 = mybir.dt.bfloat16

    nc = bacc.Bacc("TRN2", target_bir_lowering=False, debug=False)
    x_h = nc.dram_tensor("x", [BPC, NCG, 128, NBLK, HP], bf16,
                         kind="ExternalInput")
    s_h = nc.dram_tensor("s", [BPC, NCG, 128, KH, 128], bf16,
                         kind="ExternalInput")
    o_h = nc.dram_tensor("o", [BPC, NCG, 128, NBLK, H], bf16,
                         kind="ExternalOutput")
    x, s, o = x_h.ap(), s_h.ap(), o_h.ap()

    with tile.TileContext(nc) as tc:
        with (
            tc.tile_pool(name="xpool", bufs=3) as xpool,
            tc.tile_pool(name="spool", bufs=3) as spool,
            tc.tile_pool(name="outp", bufs=3) as outp,
            tc.psum_pool(name="ps", bufs=2) as ps,
        ):
            for b in range(BPC):
                for g in range(NCG):
                    xt = xpool.tile([128, NBLK, HP], bf16, name="xt")
                    st = spool.tile([128, KH, 128], bf16, name="st")
                    nc.sync.dma_start(out=xt, in_=x[b, g])
                    nc.sync.dma_start(out=st, in_=s[b, g])

                    p0 = ps.tile([128, NBLK - 1, H], f32, name="p0", tag="p0")
                    p1 = ps.tile([128, H], f32, name="p1", tag="p1")
                    for i in range(KH):
                        nc.tensor.matmul(
                            out=p0,
                            lhsT=st[:, i, :],
                            rhs=xt[:, : NBLK - 1, i : i + H],
                            start=(i == 0),
                            stop=(i == KH - 1),
                        )
                        nc.tensor.matmul(
                            out=p1,
                            lhsT=st[:, i, :],
                            rhs=xt[:, NBLK - 1, i : i + H],
                            start=(i == 0),
                            stop=(i == KH - 1),
                        )

                    ot = outp.tile([128, NBLK, H], bf16, name="ot")
                    nc.scalar.copy(ot[:, : NBLK - 1, :], p0)
                    nc.scalar.copy(ot[:, NBLK - 1, :], p1)
                    # Act HWDGE queue: halves SP-queue load (x+S stay there);
                    # cannot park -- its evac dependency just ran on Act itself
                    nc.scalar.dma_start(out=o[b, g], in_=ot)

    nc.compile()
    return nc


def _get_program():
    if "nc" not in _PROGRAM_CACHE:
        _PROGRAM_CACHE["nc"] = _build_program()
    return _PROGRAM_CACHE["nc"]


def _prep_inputs(inputs, kernels):
    """Host-side shard + layout transform. Returns per-core input maps."""
    bf16 = ml_dtypes.bfloat16
    # ---- moving tiles: [B, NCG, (ci, ws), NBLK, HP] ----------------------
    # padded image in [B, C, W, H] with W padded to cover block 4's halo
    # (w in [-3, VAL*NBLK + PAD) -> offset +3, width WPAD) and H padded +-3.
    WPAD = VAL * NBLK + 2 * PAD  # 136 >= last block start 101 + WSUB
    xp = np.zeros((B, C, WPAD, HP), dtype=bf16)
    xp[:, :, PAD : PAD + W, PAD : PAD + H] = np.transpose(
        inputs, (0, 3, 2, 1)
    ).astype(bf16)
    # blocks: block k covers padded-W slice [k*VAL, k*VAL + WSUB)
    st = xp.strides
    blocks = np.lib.stride_tricks.as_strided(
        xp,
        shape=(B, C, NBLK, WSUB, HP),
        strides=(st[0], st[1], st[2] * VAL, st[2], st[3]),
    )
    # -> [B, cg, ci, ws, blk, HP] -> [B, NCG, 128, NBLK, HP]
    xt = np.ascontiguousarray(
        blocks.reshape(B, NCG, CSUB, NBLK, WSUB, HP).transpose(0, 1, 2, 4, 3, 5)
    ).reshape(B, NCG, 128, NBLK, HP)

    # ---- stationaries: [B, NCG, p=(ci,ws), i, o=(co,wo)] -----------------
    # S[p, i, o] = (ci==co) * w[b, i, ws-wo+3, c];  j = ws-wo+3 in [0, 7)
    ws = np.arange(WSUB)[:, None]
    wo = np.arange(WSUB)[None, :]
    j = ws - wo + PAD                      # [WSUB, WSUB]
    valid = (j >= 0) & (j < KW)
    jc = np.clip(j, 0, KW - 1)
    # kernels [B, KH, KW, C] -> kt [B, C, KH, KW]
    kt = np.transpose(np.asarray(kernels), (0, 3, 1, 2))
    # bands [B, C, KH, WSUB, WSUB] = kt[b, c, i, jc[ws, wo]] * valid
    bands = (kt[:, :, :, jc] * valid).astype(bf16)
    S = np.zeros((B, NCG, CSUB, WSUB, KH, CSUB, WSUB), dtype=bf16)
    ii = np.arange(CSUB)
    # place each channel's band on the (ci == co) diagonal
    S[:, :, ii, :, :, ii] = (
        bands.reshape(B, NCG, CSUB, KH, WSUB, WSUB)
        .transpose(2, 0, 1, 4, 3, 5)[ii]
    )
    S = np.ascontiguousarray(S).reshape(B, NCG, 128, KH, 128)

    in_maps = []
    for k in range(N_CORES):
        sl = slice(k * BPC, (k + 1) * BPC)
        in_maps.append({"x": xt[sl], "s": S[sl]})
    return in_maps


def _gather_output(results):
    # o [BPC, NCG, (ci, wo), NBLK, H] bf16 per core
    full = np.concatenate([r["o"] for r in results], axis=0).reshape(
        B, NCG, CSUB, WSUB, NBLK, H
    )
    # valid outputs: block k, wo in [PAD, PAD+VAL) -> w = k*VAL + wo - PAD
    out = np.empty((B, NCG, CSUB, W, H), dtype=np.float32)
    for k in range(NBLK):
        n = min(VAL, W - k * VAL)
        out[:, :, :, k * VAL : k * VAL + n, :] = full[
            :, :, :, PAD : PAD + n, k, :
        ].astype(np.float32)
    # [B, cg, ci, W, H] -> [B, H, W, C]
    return np.ascontiguousarray(
        out.transpose(0, 4, 3, 1, 2).reshape(B, H, W, C)
    )


def run_spmd(inputs, kernels, **spmd_kwargs):
    """Run on all 8 cores; returns (output, BassKernelResults)."""
    nc = _get_program()
    in_maps = _prep_inputs(np.asarray(inputs), np.asarray(kernels))
    res = run_bass_kernel_spmd(nc, in_maps, list(range(N_CORES)), **spmd_kwargs)
    return _gather_output(res.results), res


def kernel(inputs, kernels):
    out, _ = run_spmd(inputs, kernels)
    return out


# revision 34
# speedup vs baseline: 1.0681x; 1.0681x over previous
"""Per-sample depthwise 7x7 SAME cross-correlation on 8 trn2 NeuronCores.

Problem: inputs [32,128,128,128] (B,H,W,C), kernels [32,7,7,C].
out[b,h,w,c] = sum_{i,j} inputs[b, h+i-3, w+j-3, c] * kernels[b,i,j,c]

Strategy (pure data parallel, batch sharded 4 samples/core), v2:
  All 49 taps run on the TensorEngine as BANDED-TOEPLITZ matmuls --
  ~7 MACs per moving column instead of the 1 a diagonal stationary
  gives, so the PE does the whole conv at ~7x its diag rate and the
  elementwise engines only evacuate PSUM.

  - Partition space p = (ci, ws): 4 channels x 32 width positions.
    The moving tile per (b, cgroup-of-4) is [128, 5 wblk, 134 hpad]
    bf16: five OVERLAPPING width blocks (stride 26, halo 3 each side,
    zero-padded outside the image) so no cross-block seams exist;
    only block outputs ws in [3, 29) are valid and the host discards
    the rest. H padding (3+128+3) lives in the free dim: kernel row i
    is a free-dim slice [i : i+128] -- shifts are free.
  - Stationary per (b, i, cgroup): S[p=(ci,ws), o=(co,wo)] =
    (ci==co) * w[b, i, ws-wo+3, c] -- block-diagonal of four 32x32
    7-banded Toeplitz blocks. Engines cannot write per-partition
    -offset diagonals at rate, so the 896 stationaries per core are
    PREBUILT BY THE HOST and shipped as an extra 28.7 MB input --
    DMA is the one resource with headroom (total in+S+out ~72 MB at
    ~330 GB/s ~ 215 us, balanced against ~245 us of PE).
  - Per (b, cg): 7 accumulation matmuls into PSUM P0 [128,4,128] f32
    (one full bank, N=512) + 7 into P1 [128,128] (N=128), start/stop
    framing the i-group. ldweights (128 cols, FWL-eligible) hides
    behind the 213 ns N=512 matmuls.
  - Act evacuates PSUM to bf16 SBUF (f32 accumulation throughout; only
    the final store rounds to bf16: measured ~4e-3 max rel err vs the
    f32 reference, gate 2e-2), DMA out 21 MB bf16; host casts to f32
    and reassembles valid slices.

  Rejected: on-chip Toeplitz construction (no engine can write
  per-partition-offset diagonals faster than ~1 full-tile pass per
  stationary; gather/scatter live on GpSimd at ~2 ns/elem), Winograd
  (bf16 transform conditioning blows the 2e-2 gate), FFT (freq dims
  need >128 partitions), elementwise tap lanes (engine-sum floor
  ~650 us -- the v1 kernel at 890 us was already near it), and an
  H-strip hybrid offloading bottom rows to DVE/Act/Pool in c-major
  layout (every variant, 291-430 us modeled, lost to evac/queue
  coupling: Act products delay PSUM evacs, and a DMA whose source is
  still in flight parks its queue/sequencer head; measured vs this
  design's 259 us). Keeping both HWDGE queues loaded -- x+S loads on
  SP, evac output stores on Activation -- is worth 18 us of modeled
  SP-queue congestion alone.
"""

import numpy as np
import ml_dtypes

import concourse.bass as bass
import concourse.tile as tile
from concourse import bacc, mybir
from concourse.bass_utils import run_bass_kernel_spmd

B, H, W, C = 32, 128, 128, 128
KH = KW = 7
PAD = 3
N_CORES = 8
BPC = B // N_CORES          # samples per core
HP = H + 2 * PAD            # 134 padded rows (free dim)
CSUB = 4                    # channels per partition tile
WSUB = 32                   # width positions per channel in partitions
NCG = C // CSUB             # 32 channel groups
VAL = WSUB - 2 * PAD        # 26 valid outputs per width block
NBLK = -(-W // VAL)         # 5 overlapping width blocks (stride VAL)

_PROGRAM_CACHE = {}


def _build_program():
    f32 = mybir.dt.float32
    bf16 = mybir.dt.bfloat16

    nc = bacc.Bacc("TRN2", target_bir_lowering=False, debug=False)
    x_h = nc.dram_tensor("x", [BPC, NCG, 128, NBLK, HP], bf16,
                         kind="ExternalInput")
    s_h = nc.dram_tensor("s", [BPC, NCG, 128, KH, 128], bf16,
                         kind="ExternalInput")
    o_h = nc.dram_tensor("o", [BPC, NCG, 128, NBLK, H], bf16,
                         kind="ExternalOutput")
    x, s, o = x_h.ap(), s_h.ap(), o_h.ap()

    with tile.TileContext(nc) as tc:
        with (
            tc.tile_pool(name="xpool", bufs=3) as xpool,
            tc.tile_pool(name="spool", bufs=3) as spool,
            tc.tile_pool(name="outp", bufs=3) as outp,
            tc.psum_pool(name="ps", bufs=2) as ps,
        ):
            for b in range(BPC):
                for g in range(NCG):
                    xt = xpool.tile([128, NBLK, HP], bf16, name="xt")
                    st = spool.tile([128, KH, 128], bf16, name="st")
                    nc.sync.dma_start(out=xt, in_=x[b, g])
                    nc.sync.dma_start(out=st, in_=s[b, g])

                    p0 = ps.tile([128, NBLK - 1, H], f32, name="p0", tag="p0")
                    p1 = ps.tile([128, H], f32, name="p1", tag="p1")
                    for i in range(KH):
                        nc.tensor.matmul(
                            out=p0,
                            lhsT=st[:, i, :],
                            rhs=xt[:, : NBLK - 1, i : i + H],
                            start=(i == 0),
                            stop=(i == KH - 1),
                        )
                        nc.tensor.matmul(
                            out=p1,
                            lhsT=st[:, i, :],
                            rhs=xt[:, NBLK - 1, i : i + H],
                            start=(i == 0),
                            stop=(i == KH - 1),
                        )

                    ot = outp.tile([128, NBLK, H], bf16, name="ot")
                    nc.scalar.copy(ot[:, : NBLK - 1, :], p0)
                    nc.scalar.copy(ot[:, NBLK - 1, :], p1)
                    # Act HWDGE queue: halves SP-queue load (x+S stay there);
                    # cannot park -- its evac dependency just ran on Act itself
                    nc.scalar.dma_start(out=o[b, g], in_=ot)

    nc.compile()
    return nc


def _get_program():
    if "nc" not in _PROGRAM_CACHE:
        _PROGRAM_CACHE["nc"] = _build_program()
    return _PROGRAM_CACHE["nc"]


def _prep_inputs(inputs, kernels):
    """Host-side shard + layout transform. Returns per-core input maps."""
    bf16 = ml_dtypes.bfloat16
    # ---- moving tiles: [B, NCG, (ci, ws), NBLK, HP] ----------------------
    # padded image in [B, C, W, H] with W padded to cover block 4's halo
    # (w in [-3, VAL*NBLK + PAD) -> offset +3, width WPAD) and H padded +-3.
    WPAD = VAL * NBLK + 2 * PAD  # 136 >= last block start 101 + WSUB
    xp = np.zeros((B, C, WPAD, HP), dtype=bf16)
    xp[:, :, PAD : PAD + W, PAD : PAD + H] = np.transpose(
        inputs, (0, 3, 2, 1)
    ).astype(bf16)
    # blocks: block k covers padded-W slice [k*VAL, k*VAL + WSUB)
    st = xp.strides
    blocks = np.lib.stride_tricks.as_strided(
        xp,
        shape=(B, C, NBLK, WSUB, HP),
        strides=(st[0], st[1], st[2] * VAL, st[2], st[3]),
    )
    # -> [B, cg, ci, ws, blk, HP] -> [B, NCG, 128, NBLK, HP]
    xt = np.ascontiguousarray(
        blocks.reshape(B, NCG, CSUB, NBLK, WSUB, HP).transpose(0, 1, 2, 4, 3, 5)
    ).reshape(B, NCG, 128, NBLK, HP)

    # ---- stationaries: [B, NCG, p=(ci,ws), i, o=(co,wo)] -----------------
    # S[p, i, o] = (ci==co) * w[b, i, ws-wo+3, c];  j = ws-wo+3 in [0, 7)
    ws = np.arange(WSUB)[:, None]
    wo = np.arange(WSUB)[None, :]
    j = ws - wo + PAD                      # [WSUB, WSUB]
    valid = (j >= 0) & (j < KW)
    jc = np.clip(j, 0, KW - 1)
    # kernels [B, KH, KW, C] -> kt [B, C, KH, KW]
    kt = np.transpose(np.asarray(kernels), (0, 3, 1, 2))
    # bands [B, C, KH, WSUB, WSUB] = kt[b, c, i, jc[ws, wo]] * valid
    bands = (kt[:, :, :, jc] * valid).astype(bf16)
    S = np.zeros((B, NCG, CSUB, WSUB, KH, CSUB, WSUB), dtype=bf16)
    ii = np.arange(CSUB)
    # place each channel's band on the (ci == co) diagonal
    S[:, :, ii, :, :, ii] = (
        bands.reshape(B, NCG, CSUB, KH, WSUB, WSUB)
        .transpose(2, 0, 1, 4, 3, 5)[ii]
    )
    S = np.ascontiguousarray(S).reshape(B, NCG, 128, KH, 128)

    in_maps = []
    for k in range(N_CORES):
        sl = slice(k * BPC, (k + 1) * BPC)
        in_maps.append({"x": xt[sl], "s": S[sl]})
    return in_maps


def _gather_output(results):
    # o [BPC, NCG, (ci, wo), NBLK, H] bf16 per core
    full = np.concatenate([r["o"] for r in results], axis=0).reshape(
        B, NCG, CSUB, WSUB, NBLK, H
    )
    # valid outputs: block k, wo in [PAD, PAD+VAL) -> w = k*VAL + wo - PAD
    out = np.empty((B, NCG, CSUB, W, H), dtype=np.float32)
    for k in range(NBLK):
        n = min(VAL, W - k * VAL)
        out[:, :, :, k * VAL : k * VAL + n, :] = full[
            :, :, :, PAD : PAD + n, k, :
        ].astype(np.float32)
    # [B, cg, ci, W, H] -> [B, H, W, C]
    return np.ascontiguousarray(
        out.transpose(0, 4, 3, 1, 2).reshape(B, H, W, C)
    )


def run_spmd(inputs, kernels, **spmd_kwargs):
    """Run on all 8 cores; returns (output, BassKernelResults)."""
    nc = _get_program()
    in_maps = _prep_inputs(np.asarray(inputs), np.asarray(kernels))
    res = run_bass_kernel_spmd(nc, in_maps, list(range(N_CORES)), **spmd_kwargs)
    return _gather_output(res.results), res


def kernel(inputs, kernels):
    out, _ = run_spmd(inputs, kernels)
    return out
